# revision 41
# baseline (speedup 1.0000x reference)
"""AtomToTokenCrossAttn distributed Bass kernel for 8 TRN2 NeuronCores.

Sharding: the 16384 (B*N) token rows are split into 8 contiguous shards of
2048 rows (each core owns half of one batch's tokens). Atom windows are
contiguous with stride 8 (starts = 8n), so each core only needs the atom
slice covering its tokens -- no collectives.

Structure:
  - T=15 tokens per attention tile with HOST-OVERLAPPED atom chunks
    (stride 120, width 128): every token's window [8i, 8i+16) fits its
    chunk exactly -> no spill matmuls.
  - all SBUF transposes (s_n -> sT, a_n -> aT) via the DMA XBAR
    (dma_start transpose=True) -- nothing on the PE or vector engines.
  - ragged-window mask folded into the score matmul as one-hot columns
    against ONE Toeplitz step matrix: +1 at row end-1, -1 at row 8i-1,
    +1 at the all-NEG row 127 (rows encode NEG*(p > k)).
  - softmax denominators: per-head ones-matmuls write each head's column
    sums into its own 32-partition block; normalization happens once at
    the x-stage (xb = x_psum * (rec*t1), with rec per partition-head).
    The +cv V-bias rides the wo matmul via a host-folded wo_cv term.
  - LN stats: one bn_stats per PAIR-INTERLEAVED chunk pair (even/odd
    groups = the two chunks); rstd via a Pade artanh-series ln on DVE +
    one ACT exp, keeping ACT on a single function table
    (exp/tanh/identity/copy) for the whole run.
  - two-phase schedule: phase 1 = s-pipeline in 4 quarters (stats ->
    apply -> XBAR transpose -> Q/G/qw) then the a-pipeline with
    prep(b-1) overlapping stats(b) and applies split DVE/ACT/Pool;
    phase 2 = uninterrupted attention stream with the scores matmuls of
    group g+1 emitted ahead of group g's softmax/context stages so the
    PE never waits on the ACT exp; output chunk every 4 groups.
"""

import numpy as np
import ml_dtypes

import concourse.bass as bass
import concourse.mybir as mybir
import concourse.tile as tile
from concourse import bacc
from concourse.bass_utils import run_bass_kernel_spmd

F32 = mybir.dt.float32
BF16 = mybir.dt.bfloat16
AOP = mybir.AluOpType
AFT = mybir.ActivationFunctionType
PSUM = bass.MemorySpace.PSUM

B, N, M = 4, 4096, 32768
D_TOK, D_ATOM, H, D_H = 512, 128, 4, 32
W_MAX = 16
LN_EPS = 1e-5
NC_CORES = 8
TOK = (B * N) // NC_CORES          # 2048 tokens per core
T = 15                             # tokens per attention tile
NT_FULL = TOK // T                 # 136 full tiles
T_TAIL = TOK - NT_FULL * T         # 8 tokens in tail tile
NT = NT_FULL + 1                   # 137 tiles/chunks
STRIDE = 120                       # atoms between chunk starts (8*T)
CH = NT
CH_PAD = CH + 1                    # 138: pad chunk so pairs divide evenly
NPAIR = CH_PAD // 2                # 69 interleaved chunk pairs
A_ROWS = STRIDE * (CH - 1) + 128   # 16448 local atom rows
NBLK = (CH + 15) // 16             # 9 blocks of <=16 chunks
COLS = NT_FULL * H * T + H * T_TAIL  # 8192 score columns
GCOL = 8 * H * T                   # 480 columns per full group
NG_FULL = NT_FULL // 8             # 17 full groups
NEG = -50.0

_cache = {}


def _tile_cols(t):
    if t < NT_FULL:
        return t * H * T, H * T, T
    return NT_FULL * H * T, H * T_TAIL, T_TAIL


def _build(nc):
    a8 = nc.declare_dram_parameter("a8", [128, CH_PAD * 128], BF16,
                                   isOutput=False)
    s8 = nc.declare_dram_parameter("s8", [128, 16 * 512], BF16, isOutput=False)
    r2e = nc.declare_dram_parameter("r2e", [128, COLS], BF16, isOutput=False)
    ub_e = nc.declare_dram_parameter("ub_e", [128, 128], BF16, isOutput=False)
    wq1 = nc.declare_dram_parameter("wq1", [512, 128], BF16, isOutput=False)
    wg1 = nc.declare_dram_parameter("wg1", [512, 128], BF16, isOutput=False)
    wk1t = nc.declare_dram_parameter("wk1t", [128, 512], BF16, isOutput=False)
    wv1 = nc.declare_dram_parameter("wv1", [128, 512], BF16, isOutput=False)
    wo = nc.declare_dram_parameter("wo", [128, 512], BF16, isOutput=False)
    cq = nc.declare_dram_parameter("cq", [128, 1], F32, isOutput=False)
    cv = nc.declare_dram_parameter("cv", [128, 1], F32, isOutput=False)
    cgh = nc.declare_dram_parameter("cgh", [128, 1], F32, isOutput=False)
    o_t = nc.declare_dram_parameter("o_t", [4, 128, TOK], BF16, isOutput=True)

    # a8 is pair-interleaved: [p, pair j, d, c] with chunk = 2j+c, so one
    # bn_stats over a [128, 256] slice yields both chunks' stats (even/odd)
    a8v = a8[:, :].rearrange("p (j x) -> p j x", x=256)
    s8v = s8[:, :].rearrange("p (c d) -> p c d", d=512)

    with tile.TileContext(nc) as tc:
        with tc.tile_pool(name="pp", bufs=1) as pp:
            # ---- persistent tiles
            a8s = pp.tile([128, NPAIR, 128, 2], BF16, name="a8s")
            a_n = pp.tile([128, CH, 128], BF16, name="a_n")
            aT = pp.tile([128, CH, 128], BF16, name="aT")
            sTt = pp.tile([128, 64, 128], BF16, name="sTt")
            qw = pp.tile([128, 4, TOK], BF16, name="qw")
            t1 = pp.tile([128, TOK], BF16, name="t1")
            xbuf = pp.tile([128, TOK], BF16, name="xbuf")
            ub_e_sb = pp.tile([128, 128], BF16, name="ub_e")
            ones_a = pp.tile([128, 128], BF16, name="ones")
            wq_sb = pp.tile([128, 4, 128], BF16, name="wq")
            wg_sb = pp.tile([128, 4, 128], BF16, name="wg")
            wk_sb = pp.tile([128, 4, 128], BF16, name="wk")
            wv_sb = pp.tile([128, 4, 128], BF16, name="wv")
            wo_sb = pp.tile([128, 4, 128], BF16, name="wo")
            cq_sb = pp.tile([128, 1], F32, name="cq")
            cv_sb = pp.tile([128, 1], F32, name="cv")
            cgh_sb = pp.tile([128, 1], F32, name="cgh")
            st6a = pp.tile([128, NPAIR, 6], F32, name="st6a")
            st6s = pp.tile([128, 16, 6], F32, name="st6s")
            st2s = pp.tile([128, 16, 2], F32, name="st2s")
            rstd_a = pp.tile([128, CH_PAD], F32, name="rstd_a")
            nmr_a = pp.tile([128, CH_PAD], F32, name="nmr_a")
            rstd_s = pp.tile([128, 16], F32, name="rstd_s")
            nmr_s = pp.tile([128, 16], F32, name="nmr_s")
            pd_a = pp.tile([128, CH_PAD], F32, name="pd_a")
            pd_b = pp.tile([128, CH_PAD], F32, name="pd_b")
            pd_u = pp.tile([128, CH_PAD], F32, name="pd_u")
            pd_c = pp.tile([128, CH_PAD], F32, name="pd_c")

            nc.vector.memset(ones_a, 1.0)
            s8_sb = pp.tile([128, 16, 512], BF16, name="s8")
            nc.sync.dma_start(s8_sb[:, 0:4, :], s8v[:, 0:4, :])
            # s-phase-critical weights right behind the first s8 quarter
            for dst, dsrc in (
                (wq_sb, wq1[:, :].rearrange("(c p) m -> p c m", p=128)),
                (wg_sb, wg1[:, :].rearrange("(c p) m -> p c m", p=128)),
                (wk_sb, wk1t[:, :].rearrange("k (h m) -> k h m", m=128)),
                (cq_sb, cq[:, :]),
                (cgh_sb, cgh[:, :]),
            ):
                nc.sync.dma_start(dst, dsrc)
            for q in range(1, 4):
                nc.sync.dma_start(s8_sb[:, 4 * q:4 * q + 4, :],
                                  s8v[:, 4 * q:4 * q + 4, :])
            # attention-time weights via gpsimd SWDGE (Pool is idle early)
            for dst, dsrc in (
                (wv_sb, wv1[:, :].rearrange("k (h m) -> k h m", m=128)),
                (wo_sb, wo[:, :].rearrange("k (c m) -> k c m", m=128)),
                (cv_sb, cv[:, :]),
                (ub_e_sb, ub_e[:, :]),
            ):
                nc.gpsimd.dma_start(dst, dsrc)

            def a8_dma(b):
                j0, j1 = b * 8, min(NPAIR, b * 8 + 8)
                nc.sync.dma_start(
                    a8s[:, j0:j1, :, :].rearrange("p j d c -> p j (d c)"),
                    a8v[:, j0:j1, :])

            for b in range(3):
                a8_dma(b)

            def rstd_math(var_v, mean_v, vscale, sl, rstd, nmr, shape):
                """rstd = exp(-0.5*ln(vscale*var+eps)) via Pade artanh ln;
                nmr = -mean*rstd. DVE + one ACT exp (attention table set)."""
                def v(t):
                    t = t[:, sl]
                    return t.rearrange("p (j c) -> p j c", c=2) \
                        if shape == 3 else t
                a, b, u, c = v(pd_a), v(pd_b), v(pd_u), v(pd_c)
                nc.vector.tensor_scalar(a, var_v, vscale, 1.0 + LN_EPS,
                                        AOP.mult, AOP.add)
                nc.vector.tensor_scalar(b, var_v, vscale, LN_EPS - 1.0,
                                        AOP.mult, AOP.add)
                nc.vector.reciprocal(a, a)
                nc.vector.tensor_tensor(u, b, a, AOP.mult)
                nc.vector.tensor_tensor(c, u, u, AOP.mult)
                nc.vector.tensor_scalar(c, c, 1.0 / 3.0, 1.0,
                                        AOP.mult, AOP.add)
                nc.vector.tensor_tensor(u, u, c, AOP.mult)
                rv = rstd[:, sl].rearrange("p (j c) -> p j c", c=2) \
                    if shape == 3 else rstd[:, sl]
                nv = nmr[:, sl].rearrange("p (j c) -> p j c", c=2) \
                    if shape == 3 else nmr[:, sl]
                nc.scalar.activation(rv, u, AFT.Exp, scale=-1.0)
                nc.vector.tensor_tensor(nv, mean_v, rv, AOP.mult)
                nc.vector.tensor_scalar(nv, nv, -1.0, None, AOP.mult)

            # =================== a stats (per block of 8 pairs) ==========
            def a_stats(b):
                j0, j1 = b * 8, min(NPAIR, b * 8 + 8)
                for j in range(j0, j1):
                    nc.vector.bn_stats(
                        st6a[:, j, :],
                        a8s[:, j, :, :].rearrange("p d c -> p (d c)"))
                sl = slice(2 * j0, 2 * j1)
                stv = st6a[:, j0:j1, :].rearrange("p j (c s) -> p j c s", s=3)
                rstd_math(stv[:, :, :, 2], stv[:, :, :, 1], 1.0 / 128.0,
                          sl, rstd_a, nmr_a, 3)

            # ============ unified s-phase + attention pipeline ===========
            with (
                tc.tile_pool(name="ew", bufs=2) as ew,
                tc.tile_pool(name="r2p", bufs=3) as r2p,
                tc.tile_pool(name="psSC", bufs=2, space=PSUM) as psSC,
                tc.tile_pool(name="psDN", bufs=1, space=PSUM) as psDN,
                tc.tile_pool(name="psCT", bufs=2, space=PSUM) as psCT,
                tc.tile_pool(name="psX", bufs=1, space=PSUM) as psX,
                tc.tile_pool(name="psO", bufs=2, space=PSUM) as psO,
            ):
                def s_quarter(q):
                    cs = slice(4 * q, 4 * q + 4)
                    for c in range(4 * q, 4 * q + 4):
                        nc.vector.bn_stats(st6s[:, c, :], s8_sb[:, c, :])
                        nc.vector.bn_aggr(st2s[:, c, :], st6s[:, c, :])
                    rstd_math(st2s[:, cs, 1], st2s[:, cs, 0], 1.0,
                              cs, rstd_s, nmr_s, 2)
                    s_n = ew.tile([128, 4, 512], BF16, tag="sn", name="sn")
                    for j, c in enumerate(range(4 * q, 4 * q + 4)):
                        if j == 0:
                            nc.vector.tensor_scalar(
                                s_n[:, j, :], s8_sb[:, c, :],
                                rstd_s[:, c:c + 1], nmr_s[:, c:c + 1],
                                AOP.mult, AOP.add)
                        elif j == 1:
                            nc.gpsimd.tensor_scalar(
                                s_n[:, j, :], s8_sb[:, c, :],
                                rstd_s[:, c:c + 1], nmr_s[:, c:c + 1],
                                AOP.mult, AOP.add)
                        else:
                            nc.scalar.activation(
                                s_n[:, j, :], s8_sb[:, c, :], AFT.Identity,
                                bias=nmr_s[:, c:c + 1],
                                scale=rstd_s[:, c:c + 1])
                    nc.sync.dma_start(
                        sTt[:, 16 * q:16 * q + 16, :],
                        s_n[:, :, :].rearrange("p c d -> p (c d)"),
                        transpose=True)
                    tsl = slice(512 * q, 512 * (q + 1))
                    qt = ew.tile([128, 512], BF16, tag="qt", name="qt")
                    ps_q = psSC.tile([128, 512], F32, tag="sc", name="q")
                    for k in range(4):
                        nc.tensor.matmul(ps_q, wq_sb[:, k, :],
                                         sTt[:, 16 * q + k:16 * q + 16:4, :],
                                         start=(k == 0), stop=(k == 3))
                    nc.scalar.activation(qt, ps_q, AFT.Identity, bias=cq_sb)
                    ps_g = psSC.tile([128, 512], F32, tag="sc", name="g")
                    for k in range(4):
                        nc.tensor.matmul(ps_g, wg_sb[:, k, :],
                                         sTt[:, 16 * q + k:16 * q + 16:4, :],
                                         start=(k == 0), stop=(k == 3))
                    # t1 = 1 + tanh(0.5*G + 0.5*cg); w_o carries the 0.5
                    nc.scalar.activation(t1[:, tsl], ps_g, AFT.Tanh,
                                         bias=cgh_sb, scale=0.5)
                    nc.vector.tensor_scalar(t1[:, tsl], t1[:, tsl], 1.0,
                                            None, AOP.add)
                    for h in range(H):
                        qwp = psCT.tile([128, 512], F32, tag="ctx", name="w")
                        nc.tensor.matmul(qwp, wk_sb[:, h, :], qt,
                                         start=True, stop=True)
                        nc.scalar.activation(qw[:, h, tsl], qwp, AFT.Copy)

                r2e_tiles = {}

                def r2e_fetch(g):
                    if g > NG_FULL:
                        return
                    col0 = _tile_cols(8 * g if g < NG_FULL else NT_FULL)[0]
                    ncol = GCOL if g < NG_FULL else H * T_TAIL
                    t = r2p.tile([128, GCOL], BF16, tag="r2e", name="r2e")
                    nc.sync.dma_start(t[:, :ncol], r2e[:, col0:col0 + ncol])
                    r2e_tiles[g] = t

                def g_meta(g):
                    if g < NG_FULL:
                        tiles = list(range(8 * g, 8 * g + 8))
                    else:
                        tiles = [NT_FULL]
                    ncol = sum(_tile_cols(t)[1] for t in tiles)
                    ntok = sum(_tile_cols(t)[2] for t in tiles)
                    return tiles, ncol, ntok, ncol // (H * len(tiles))

                sc_tiles = {}

                def g_scores(g):
                    tiles, ncol, ntok, tw = g_meta(g)
                    sc = psSC.tile([128, 512], F32, tag="sc",
                                   name="sc")[:, :ncol]
                    sc_tiles[g] = sc
                    nc.tensor.matmul(sc, ub_e_sb,
                                     r2e_tiles.pop(g)[:, :ncol],
                                     start=True, stop=False)
                    r2e_fetch(g + 2)
                    for j, t in enumerate(tiles):
                        csl = slice(j * H * tw, (j + 1) * H * tw)
                        qv = qw[:, :, t * T:t * T + tw]
                        nc.tensor.matmul(sc[:, csl], aT[:, t, :], qv,
                                         start=False, stop=True,
                                         skip_group_check=True)

                def g_rest(g):
                    tiles, ncol, ntok, tw = g_meta(g)
                    tok0 = tiles[0] * T
                    sc = sc_tiles.pop(g)
                    exp_a = ew.tile([128, GCOL], BF16, tag="exp",
                                    name="exp")[:, :ncol]
                    nc.scalar.activation(exp_a, sc, AFT.Exp)
                    expv = exp_a.rearrange("p (t h i) -> p t h i", h=H, i=tw)
                    dn = psDN.tile([128, GCOL // 4], F32, tag="dn",
                                   name="dn")[:, :ntok]
                    dnv = dn.rearrange("p (t i) -> p t i", i=tw)
                    for h in range(H):
                        nc.tensor.matmul(dnv[32 * h:32 * h + 32],
                                         ones_a[:, :32], expv[:, :, h, :],
                                         start=True, stop=True,
                                         skip_group_check=True,
                                         tile_position=(0, 32 * h))
                    rec = ew.tile([128, GCOL // 4], F32, tag="rec",
                                  name="rec")[:, :ntok]
                    nc.vector.reciprocal_approx_fast(rec, dn)
                    tsl = slice(tok0, tok0 + ntok)
                    t1r = ew.tile([128, GCOL // 4], F32, tag="t1r",
                                  name="t1r")[:, :ntok]
                    nc.gpsimd.tensor_tensor(t1r, rec, t1[:, tsl], AOP.mult)
                    ctx = psCT.tile([128, 512], F32, tag="ctx",
                                    name="ctx")[:, :ncol]
                    for j, t in enumerate(tiles):
                        csl = slice(j * H * tw, (j + 1) * H * tw)
                        nc.tensor.matmul(ctx[:, csl], a_n[:, t, :],
                                         exp_a[:, csl], start=True,
                                         stop=True, skip_group_check=True)
                    ctx_sb = ew.tile([128, GCOL], BF16, tag="ctxs",
                                     name="ctxs")[:, :ncol]
                    nc.scalar.activation(ctx_sb, ctx, AFT.Copy)
                    ctxv = ctx_sb.rearrange("p (t h i) -> p t h i", h=H, i=tw)
                    x_ps = psX.tile([128, GCOL // 4], F32, tag="x",
                                    name="x")[:, :ntok]
                    for h in range(H):
                        nc.tensor.matmul(x_ps, wv_sb[:, h, :],
                                         ctxv[:, :, h, :],
                                         start=(h == 0), stop=(h == 3),
                                         skip_group_check=True)
                    # xb = x*rec*t1 + cv*t1 (full gated output incl V-bias)
                    u2 = ew.tile([128, GCOL // 4], F32, tag="u2",
                                 name="u2")[:, :ntok]
                    nc.vector.tensor_tensor(u2, x_ps, t1r, AOP.mult)
                    nc.vector.scalar_tensor_tensor(xbuf[:, tsl], t1[:, tsl],
                                                   cv_sb, u2, AOP.mult,
                                                   AOP.add)

                def wo_chunk(w):
                    w0 = 480 * w
                    L = 480 if w < 4 else TOK - w0
                    for c in range(4):
                        ps_o = psO.tile([128, 512], F32, tag="o",
                                        name="o")[:, :L]
                        nc.tensor.matmul(ps_o, wo_sb[:, c, :],
                                         xbuf[:, w0:w0 + L],
                                         start=True, stop=True)
                        ot = ew.tile([128, GCOL], BF16, tag="ot",
                                     name="ot")[:, :L]
                        if c % 2:
                            nc.scalar.activation(ot, ps_o, AFT.Copy)
                        else:
                            nc.vector.tensor_copy(ot, ps_o)
                        nc.sync.dma_start(o_t[c, :, w0:w0 + L], ot)

                def prep(b):
                    """LN-apply block b's chunks and XBAR-transpose to aT."""
                    c0, c1 = b * 16, min(CH, b * 16 + 16)
                    for m, c in enumerate(range(c0, c1)):
                        src = a8s[:, c // 2, :, c % 2]
                        r = m % 16
                        if r < 2:
                            nc.vector.tensor_scalar(
                                a_n[:, c, :], src,
                                rstd_a[:, c:c + 1], nmr_a[:, c:c + 1],
                                AOP.mult, AOP.add)
                        elif r < 9:
                            nc.scalar.activation(
                                a_n[:, c, :], src, AFT.Identity,
                                bias=nmr_a[:, c:c + 1],
                                scale=rstd_a[:, c:c + 1])
                        else:
                            nc.gpsimd.tensor_scalar(
                                a_n[:, c, :], src,
                                rstd_a[:, c:c + 1], nmr_a[:, c:c + 1],
                                AOP.mult, AOP.add)
                    nc.sync.dma_start(
                        aT[:, c0:c1, :],
                        a_n[:, c0:c1, :].rearrange("p c d -> p (c d)"),
                        transpose=True)

                # software pipeline: s-phase quarters with a-stats/prep
                # interleaved; then PREP two blocks ahead of its groups,
                # STATS three ahead, a8 DMA three ahead
                for q in range(4):
                    s_quarter(q)
                # phase 1: full a pipeline; prep(b-1) overlaps stats(b)
                a_stats(0)
                for b in range(1, NBLK):
                    if b + 2 < NBLK:
                        a8_dma(b + 2)
                    a_stats(b)
                    prep(b - 1)
                    if b == 5:
                        r2e_fetch(0)
                        r2e_fetch(1)
                    if b == 7:
                        g_scores(0)
                prep(NBLK - 1)
                # phase 2: uninterrupted attention group stream
                for g in range(NG_FULL + 1):
                    if g + 1 <= NG_FULL:
                        g_scores(g + 1)
                    g_rest(g)
                    if (g + 1) % 4 == 0:
                        wo_chunk((g + 1) // 4 - 1)
                wo_chunk(4)
    nc.compile()
    nc.finalize()
    return nc


def _prep(s, a, starts, counts, token_mask, w_q, w_k, w_v, w_g, w_o,
          ln_q_g, ln_q_b, ln_kv_g, ln_kv_b):
    bf = ml_dtypes.bfloat16
    sc = 1.0 / np.sqrt(np.float32(D_H))
    wq1 = ((ln_q_g[:, None] * w_q) * sc).astype(bf)
    wg1 = (ln_q_g[:, None] * w_g).astype(bf)
    wk1_t = np.asarray((ln_kv_g[:, None] * w_k).T, np.float32)
    wk1t = np.zeros((128, 4 * 128), np.float32)
    wv1_f = np.asarray(ln_kv_g[:, None] * w_v, np.float32)
    wv1 = np.zeros((128, 4 * 128), np.float32)
    for h in range(4):
        wk1t[h * 32:(h + 1) * 32, h * 128:(h + 1) * 128] = \
            wk1_t[h * 32:(h + 1) * 32, :]
        wv1[:, h * 128:(h + 1) * 128] = wv1_f * \
            (np.arange(128)[None, :] // 32 == h)
    wk1t = wk1t.astype(bf)
    wv1 = wv1.astype(bf)
    cq = ((ln_q_b @ w_q) * sc).astype(np.float32).reshape(128, 1)
    cgh = (0.5 * (ln_q_b @ w_g)).astype(np.float32).reshape(128, 1)
    cv = (ln_kv_b @ w_v).astype(np.float32).reshape(128, 1)  # V bias
    wo_h = (0.5 * np.asarray(w_o, np.float32)).astype(bf)

    jj = np.arange(128)
    ub_e = (NEG * (jj[None, :] > jj[:, None])).astype(np.float32)  # p > k
    ub_e[127, :] = NEG  # row 127 is otherwise all-zero: constant-NEG row
    ub_e = ub_e.astype(bf)

    j_tok = np.arange(TOK)
    tile_i = np.where(j_tok < NT_FULL * T, j_tok % T, j_tok - NT_FULL * T)
    col_base = np.where(j_tok < NT_FULL * T, (j_tok // T) * H * T,
                        NT_FULL * H * T)
    tw = np.where(j_tok < NT_FULL * T, T, T_TAIL)

    in_maps = []
    for c in range(NC_CORES):
        b, half = c // 2, c % 2
        n0 = half * TOK
        st = np.asarray(starts[b, n0:n0 + TOK], np.int64)
        ct = np.asarray(counts[b, n0:n0 + TOK], np.int64)
        lo = int(st[0])
        st_loc = st - lo
        assert np.all(st_loc == 8 * j_tok), "v4 premise: starts = 8n"
        assert ct.min() >= 1 and ct.max() <= W_MAX

        rows_pad = STRIDE * (CH_PAD - 1) + 128
        a_loc = np.zeros((rows_pad, 128), np.float32)
        hi = min(lo + A_ROWS, M)
        a_loc[:hi - lo] = np.asarray(a[b, lo:hi, :], np.float32)
        # pair-interleaved: a8[p, j, d, c] = a_loc[120*(2j+c) + p, d]
        idx = (STRIDE * np.arange(CH_PAD).reshape(NPAIR, 2)[None, :, :]
               + np.arange(128)[:, None, None])
        a8 = a_loc[idx].transpose(0, 1, 3, 2) \
            .reshape(128, CH_PAD * 128).astype(bf)

        s_sl = np.asarray(s[b, n0:n0 + TOK, :], np.float32)
        s8 = s_sl.reshape(16, 128, 512).transpose(1, 0, 2) \
            .reshape(128, 16 * 512).astype(bf)

        # mask = NEG*(p >= end)[end<=127] + NEG*(p < 8i), the latter as
        # const-NEG (row 127) minus NEG*(p >= 8i) via a -1 at row 8i-1
        r2 = np.zeros((128, COLS), np.float32)
        end = 8 * tile_i + ct
        for h in range(H):
            cols = col_base + h * tw + tile_i
            m_e = end <= 127
            np.add.at(r2, (np.where(m_e, end - 1, 0), cols),
                      np.where(m_e, 1.0, 0.0))
            m_s = tile_i > 0
            np.add.at(r2, (np.where(m_s, 8 * tile_i - 1, 0), cols),
                      np.where(m_s, -1.0, 0.0))
            np.add.at(r2, (np.full(TOK, 127), cols),
                      np.where(m_s, 1.0, 0.0))
        in_maps.append({
            "a8": a8, "s8": s8, "r2e": r2.astype(bf),
            "ub_e": ub_e,
            "wq1": wq1, "wg1": wg1, "wk1t": wk1t, "wv1": wv1,
            "wo": wo_h, "cv": cv, "cq": cq, "cgh": cgh,
        })
    return in_maps


def kernel(s, a, token_atom_starts, token_atom_counts, token_mask,
           w_q, w_k, w_v, w_g, w_o, ln_q_g, ln_q_b, ln_kv_g, ln_kv_b,
           trace=False):
    args = [np.asarray(x) for x in
            (s, a, token_atom_starts, token_atom_counts, token_mask,
             w_q, w_k, w_v, w_g, w_o, ln_q_g, ln_q_b, ln_kv_g, ln_kv_b)]
    in_maps = _prep(*args)
    if "nc" not in _cache:
        nc = bacc.Bacc(None, target_bir_lowering=False)
        _cache["nc"] = _build(nc)
    nc = _cache["nc"]
    res = run_bass_kernel_spmd(nc, in_maps, list(range(NC_CORES)),
                               trace=trace)
    out = np.zeros((B, N, D_TOK), np.float32)
    for c in range(NC_CORES):
        b, half = c // 2, c % 2
        n0 = half * TOK
        ot = np.asarray(res.results[c]["o_t"], np.float32)  # [4, 128, TOK]
        tm = np.asarray(args[4][b, n0:n0 + TOK], np.float32)
        out[b, n0:n0 + TOK, :] = ot.reshape(512, TOK).T * tm[:, None]
    kernel.last_exec_time_ns = res.exec_time_ns
    return out


# revision 42
# speedup vs baseline: 1.0477x; 1.0477x over previous
"""AtomToTokenCrossAttn distributed Bass kernel for 8 TRN2 NeuronCores.

Sharding: the 16384 (B*N) token rows are split into 8 contiguous shards of
2048 rows (each core owns half of one batch's tokens). Atom windows are
contiguous with stride 8 (starts = 8n), so each core only needs the atom
slice covering its tokens -- no collectives.

Structure:
  - T=15 tokens per attention tile with HOST-OVERLAPPED atom chunks
    (stride 120, width 128): every token's window [8i, 8i+16) fits its
    chunk exactly -> no spill matmuls.
  - all SBUF transposes (s_n -> sT, a_n -> aT) via the DMA XBAR
    (dma_start transpose=True) -- nothing on the PE or vector engines.
  - ragged-window mask folded into the score matmul as one-hot columns
    against ONE Toeplitz step matrix: +1 at row end-1, -1 at row 8i-1,
    +1 at the all-NEG row 127 (rows encode NEG*(p > k)).
  - softmax denominators: per-head ones-matmuls write each head's column
    sums into its own 32-partition block; normalization happens once at
    the x-stage (xb = x_psum * (rec*t1), with rec per partition-head).
    The +cv V-bias rides the wo matmul via a host-folded wo_cv term.
  - LN stats: one bn_stats per PAIR-INTERLEAVED chunk pair (even/odd
    groups = the two chunks); rstd via a Pade artanh-series ln on DVE +
    one ACT exp, keeping ACT on a single function table
    (exp/tanh/identity/copy) for the whole run.
  - two-phase schedule: phase 1 = s-pipeline in 4 quarters (stats ->
    apply -> XBAR transpose -> Q/G/qw) then the a-pipeline with
    prep(b-1) overlapping stats(b) and applies split DVE/ACT/Pool;
    phase 2 = uninterrupted attention stream with the scores matmuls of
    group g+1 emitted ahead of group g's softmax/context stages so the
    PE never waits on the ACT exp; output chunk every 4 groups.
"""

import numpy as np
import ml_dtypes

import concourse.bass as bass
import concourse.mybir as mybir
import concourse.tile as tile
from concourse import bacc
from concourse.bass_utils import run_bass_kernel_spmd

F32 = mybir.dt.float32
BF16 = mybir.dt.bfloat16
AOP = mybir.AluOpType
AFT = mybir.ActivationFunctionType
PSUM = bass.MemorySpace.PSUM

B, N, M = 4, 4096, 32768
D_TOK, D_ATOM, H, D_H = 512, 128, 4, 32
W_MAX = 16
LN_EPS = 1e-5
NC_CORES = 8
TOK = (B * N) // NC_CORES          # 2048 tokens per core
T = 15                             # tokens per attention tile
NT_FULL = TOK // T                 # 136 full tiles
T_TAIL = TOK - NT_FULL * T         # 8 tokens in tail tile
NT = NT_FULL + 1                   # 137 tiles/chunks
STRIDE = 120                       # atoms between chunk starts (8*T)
CH = NT
CH_PAD = CH + 1                    # 138: pad chunk so pairs divide evenly
NPAIR = CH_PAD // 2                # 69 interleaved chunk pairs
A_ROWS = STRIDE * (CH - 1) + 128   # 16448 local atom rows
NBLK = (CH + 15) // 16             # 9 blocks of <=16 chunks
COLS = NT_FULL * H * T + H * T_TAIL  # 8192 score columns
GCOL = 8 * H * T                   # 480 columns per full group
NG_FULL = NT_FULL // 8             # 17 full groups
NEG = -50.0

_cache = {}


def _tile_cols(t):
    if t < NT_FULL:
        return t * H * T, H * T, T
    return NT_FULL * H * T, H * T_TAIL, T_TAIL


def _build(nc):
    a8 = nc.declare_dram_parameter("a8", [128, CH_PAD * 128], BF16,
                                   isOutput=False)
    s8 = nc.declare_dram_parameter("s8", [128, 16 * 512], BF16, isOutput=False)
    r2e = nc.declare_dram_parameter("r2e", [128, COLS], BF16, isOutput=False)
    ub_e = nc.declare_dram_parameter("ub_e", [128, 128], BF16, isOutput=False)
    wq1 = nc.declare_dram_parameter("wq1", [512, 128], BF16, isOutput=False)
    wg1 = nc.declare_dram_parameter("wg1", [512, 128], BF16, isOutput=False)
    wk1t = nc.declare_dram_parameter("wk1t", [128, 512], BF16, isOutput=False)
    wv1 = nc.declare_dram_parameter("wv1", [128, 512], BF16, isOutput=False)
    wo = nc.declare_dram_parameter("wo", [128, 512], BF16, isOutput=False)
    wocv = nc.declare_dram_parameter("wocv", [128, 512], BF16, isOutput=False)
    cq = nc.declare_dram_parameter("cq", [128, 1], F32, isOutput=False)
    cgh = nc.declare_dram_parameter("cgh", [128, 1], F32, isOutput=False)
    o_t = nc.declare_dram_parameter("o_t", [4, 128, TOK], BF16, isOutput=True)

    # a8 is pair-interleaved: [p, pair j, d, c] with chunk = 2j+c, so one
    # bn_stats over a [128, 256] slice yields both chunks' stats (even/odd)
    a8v = a8[:, :].rearrange("p (j x) -> p j x", x=256)
    s8v = s8[:, :].rearrange("p (c d) -> p c d", d=512)

    with tile.TileContext(nc) as tc:
        with tc.tile_pool(name="pp", bufs=1) as pp:
            # ---- persistent tiles
            a8s = pp.tile([128, NPAIR, 128, 2], BF16, name="a8s")
            a_n = pp.tile([128, CH, 128], BF16, name="a_n")
            aT = pp.tile([128, CH, 128], BF16, name="aT")
            sTt = pp.tile([128, 64, 128], BF16, name="sTt")
            qw = pp.tile([128, 4, TOK], BF16, name="qw")
            t1 = pp.tile([128, TOK], BF16, name="t1")
            xbuf = pp.tile([128, TOK], BF16, name="xbuf")
            ub_e_sb = pp.tile([128, 128], BF16, name="ub_e")
            ones_a = pp.tile([128, 128], BF16, name="ones")
            wq_sb = pp.tile([128, 4, 128], BF16, name="wq")
            wg_sb = pp.tile([128, 4, 128], BF16, name="wg")
            wk_sb = pp.tile([128, 4, 128], BF16, name="wk")
            wv_sb = pp.tile([128, 4, 128], BF16, name="wv")
            wo_sb = pp.tile([128, 4, 128], BF16, name="wo")
            wocv_sb = pp.tile([128, 4, 128], BF16, name="wocv")
            cq_sb = pp.tile([128, 1], F32, name="cq")
            cgh_sb = pp.tile([128, 1], F32, name="cgh")
            st6a = pp.tile([128, NPAIR, 6], F32, name="st6a")
            st6s = pp.tile([128, 16, 6], F32, name="st6s")
            st2s = pp.tile([128, 16, 2], F32, name="st2s")
            rstd_a = pp.tile([128, CH_PAD], F32, name="rstd_a")
            nmr_a = pp.tile([128, CH_PAD], F32, name="nmr_a")
            rstd_s = pp.tile([128, 16], F32, name="rstd_s")
            nmr_s = pp.tile([128, 16], F32, name="nmr_s")
            pd_a = pp.tile([128, CH_PAD], F32, name="pd_a")
            pd_b = pp.tile([128, CH_PAD], F32, name="pd_b")
            pd_u = pp.tile([128, CH_PAD], F32, name="pd_u")
            pd_c = pp.tile([128, CH_PAD], F32, name="pd_c")

            nc.vector.memset(ones_a, 1.0)
            s8_sb = pp.tile([128, 16, 512], BF16, name="s8")
            nc.sync.dma_start(s8_sb[:, 0:4, :], s8v[:, 0:4, :])
            # s-phase-critical weights right behind the first s8 quarter
            for dst, dsrc in (
                (wq_sb, wq1[:, :].rearrange("(c p) m -> p c m", p=128)),
                (wg_sb, wg1[:, :].rearrange("(c p) m -> p c m", p=128)),
                (wk_sb, wk1t[:, :].rearrange("k (h m) -> k h m", m=128)),
                (cq_sb, cq[:, :]),
                (cgh_sb, cgh[:, :]),
            ):
                nc.sync.dma_start(dst, dsrc)
            for q in range(1, 4):
                nc.sync.dma_start(s8_sb[:, 4 * q:4 * q + 4, :],
                                  s8v[:, 4 * q:4 * q + 4, :])
            # attention-time weights via gpsimd SWDGE (Pool is idle early)
            for dst, dsrc in (
                (wv_sb, wv1[:, :].rearrange("k (h m) -> k h m", m=128)),
                (wo_sb, wo[:, :].rearrange("k (c m) -> k c m", m=128)),
                (wocv_sb, wocv[:, :].rearrange("k (c m) -> k c m", m=128)),
                (ub_e_sb, ub_e[:, :]),
            ):
                nc.gpsimd.dma_start(dst, dsrc)

            def a8_dma(b):
                j0, j1 = b * 8, min(NPAIR, b * 8 + 8)
                nc.sync.dma_start(
                    a8s[:, j0:j1, :, :].rearrange("p j d c -> p j (d c)"),
                    a8v[:, j0:j1, :])

            for b in range(3):
                a8_dma(b)

            def rstd_math(var_v, mean_v, vscale, sl, rstd, nmr, shape):
                """rstd = exp(-0.5*ln(vscale*var+eps)) via Pade artanh ln;
                nmr = -mean*rstd. DVE + one ACT exp (attention table set)."""
                def v(t):
                    t = t[:, sl]
                    return t.rearrange("p (j c) -> p j c", c=2) \
                        if shape == 3 else t
                a, b, u, c = v(pd_a), v(pd_b), v(pd_u), v(pd_c)
                nc.vector.tensor_scalar(a, var_v, vscale, 1.0 + LN_EPS,
                                        AOP.mult, AOP.add)
                nc.vector.tensor_scalar(b, var_v, vscale, LN_EPS - 1.0,
                                        AOP.mult, AOP.add)
                nc.vector.reciprocal(a, a)
                nc.vector.tensor_tensor(u, b, a, AOP.mult)
                nc.vector.tensor_tensor(c, u, u, AOP.mult)
                nc.vector.tensor_scalar(c, c, 1.0 / 3.0, 1.0,
                                        AOP.mult, AOP.add)
                nc.vector.tensor_tensor(u, u, c, AOP.mult)
                rv = rstd[:, sl].rearrange("p (j c) -> p j c", c=2) \
                    if shape == 3 else rstd[:, sl]
                nv = nmr[:, sl].rearrange("p (j c) -> p j c", c=2) \
                    if shape == 3 else nmr[:, sl]
                nc.scalar.activation(rv, u, AFT.Exp, scale=-1.0)
                nc.vector.tensor_tensor(nv, mean_v, rv, AOP.mult)
                nc.vector.tensor_scalar(nv, nv, -1.0, None, AOP.mult)

            # =================== a stats (per block of 8 pairs) ==========
            def a_stats(b):
                j0, j1 = b * 8, min(NPAIR, b * 8 + 8)
                for j in range(j0, j1):
                    nc.vector.bn_stats(
                        st6a[:, j, :],
                        a8s[:, j, :, :].rearrange("p d c -> p (d c)"))
                sl = slice(2 * j0, 2 * j1)
                stv = st6a[:, j0:j1, :].rearrange("p j (c s) -> p j c s", s=3)
                rstd_math(stv[:, :, :, 2], stv[:, :, :, 1], 1.0 / 128.0,
                          sl, rstd_a, nmr_a, 3)

            # ============ unified s-phase + attention pipeline ===========
            with (
                tc.tile_pool(name="ew", bufs=2) as ew,
                tc.tile_pool(name="r2p", bufs=3) as r2p,
                tc.tile_pool(name="psSC", bufs=2, space=PSUM) as psSC,
                tc.tile_pool(name="psDN", bufs=1, space=PSUM) as psDN,
                tc.tile_pool(name="psCT", bufs=2, space=PSUM) as psCT,
                tc.tile_pool(name="psX", bufs=1, space=PSUM) as psX,
                tc.tile_pool(name="psO", bufs=2, space=PSUM) as psO,
            ):
                def s_quarter(q):
                    cs = slice(4 * q, 4 * q + 4)
                    for c in range(4 * q, 4 * q + 4):
                        nc.vector.bn_stats(st6s[:, c, :], s8_sb[:, c, :])
                        nc.vector.bn_aggr(st2s[:, c, :], st6s[:, c, :])
                    rstd_math(st2s[:, cs, 1], st2s[:, cs, 0], 1.0,
                              cs, rstd_s, nmr_s, 2)
                    s_n = ew.tile([128, 4, 512], BF16, tag="sn", name="sn")
                    for j, c in enumerate(range(4 * q, 4 * q + 4)):
                        if j == 0:
                            nc.vector.tensor_scalar(
                                s_n[:, j, :], s8_sb[:, c, :],
                                rstd_s[:, c:c + 1], nmr_s[:, c:c + 1],
                                AOP.mult, AOP.add)
                        elif j == 1:
                            nc.gpsimd.tensor_scalar(
                                s_n[:, j, :], s8_sb[:, c, :],
                                rstd_s[:, c:c + 1], nmr_s[:, c:c + 1],
                                AOP.mult, AOP.add)
                        else:
                            nc.scalar.activation(
                                s_n[:, j, :], s8_sb[:, c, :], AFT.Identity,
                                bias=nmr_s[:, c:c + 1],
                                scale=rstd_s[:, c:c + 1])
                    nc.sync.dma_start(
                        sTt[:, 16 * q:16 * q + 16, :],
                        s_n[:, :, :].rearrange("p c d -> p (c d)"),
                        transpose=True)
                    tsl = slice(512 * q, 512 * (q + 1))
                    qt = ew.tile([128, 512], BF16, tag="qt", name="qt")
                    ps_q = psSC.tile([128, 512], F32, tag="sc", name="q")
                    for k in range(4):
                        nc.tensor.matmul(ps_q, wq_sb[:, k, :],
                                         sTt[:, 16 * q + k:16 * q + 16:4, :],
                                         start=(k == 0), stop=(k == 3))
                    nc.scalar.activation(qt, ps_q, AFT.Identity, bias=cq_sb)
                    ps_g = psSC.tile([128, 512], F32, tag="sc", name="g")
                    for k in range(4):
                        nc.tensor.matmul(ps_g, wg_sb[:, k, :],
                                         sTt[:, 16 * q + k:16 * q + 16:4, :],
                                         start=(k == 0), stop=(k == 3))
                    # t1 = 1 + tanh(0.5*G + 0.5*cg); w_o carries the 0.5
                    nc.scalar.activation(t1[:, tsl], ps_g, AFT.Tanh,
                                         bias=cgh_sb, scale=0.5)
                    nc.vector.tensor_scalar(t1[:, tsl], t1[:, tsl], 1.0,
                                            None, AOP.add)
                    for h in range(H):
                        qwp = psCT.tile([128, 512], F32, tag="ctx", name="w")
                        nc.tensor.matmul(qwp, wk_sb[:, h, :], qt,
                                         start=True, stop=True)
                        nc.scalar.activation(qw[:, h, tsl], qwp, AFT.Copy)

                r2e_tiles = {}

                def r2e_fetch(g):
                    if g > NG_FULL:
                        return
                    col0 = _tile_cols(8 * g if g < NG_FULL else NT_FULL)[0]
                    ncol = GCOL if g < NG_FULL else H * T_TAIL
                    t = r2p.tile([128, GCOL], BF16, tag="r2e", name="r2e")
                    nc.sync.dma_start(t[:, :ncol], r2e[:, col0:col0 + ncol])
                    r2e_tiles[g] = t

                def g_meta(g):
                    if g < NG_FULL:
                        tiles = list(range(8 * g, 8 * g + 8))
                    else:
                        tiles = [NT_FULL]
                    ncol = sum(_tile_cols(t)[1] for t in tiles)
                    ntok = sum(_tile_cols(t)[2] for t in tiles)
                    return tiles, ncol, ntok, ncol // (H * len(tiles))

                sc_tiles = {}

                def g_scores(g):
                    tiles, ncol, ntok, tw = g_meta(g)
                    sc = psSC.tile([128, 512], F32, tag="sc",
                                   name="sc")[:, :ncol]
                    sc_tiles[g] = sc
                    nc.tensor.matmul(sc, ub_e_sb,
                                     r2e_tiles.pop(g)[:, :ncol],
                                     start=True, stop=False)
                    r2e_fetch(g + 2)
                    for j, t in enumerate(tiles):
                        csl = slice(j * H * tw, (j + 1) * H * tw)
                        qv = qw[:, :, t * T:t * T + tw]
                        nc.tensor.matmul(sc[:, csl], aT[:, t, :], qv,
                                         start=False, stop=True,
                                         skip_group_check=True)

                def g_rest(g):
                    tiles, ncol, ntok, tw = g_meta(g)
                    tok0 = tiles[0] * T
                    sc = sc_tiles.pop(g)
                    exp_a = ew.tile([128, GCOL], BF16, tag="exp",
                                    name="exp")[:, :ncol]
                    nc.scalar.activation(exp_a, sc, AFT.Exp)
                    expv = exp_a.rearrange("p (t h i) -> p t h i", h=H, i=tw)
                    dn = psDN.tile([128, GCOL // 4], F32, tag="dn",
                                   name="dn")[:, :ntok]
                    dnv = dn.rearrange("p (t i) -> p t i", i=tw)
                    for h in range(H):
                        nc.tensor.matmul(dnv[32 * h:32 * h + 32],
                                         ones_a[:, :32], expv[:, :, h, :],
                                         start=True, stop=True,
                                         skip_group_check=True,
                                         tile_position=(0, 32 * h))
                    rec = ew.tile([128, GCOL // 4], F32, tag="rec",
                                  name="rec")[:, :ntok]
                    nc.vector.reciprocal_approx_fast(rec, dn)
                    tsl = slice(tok0, tok0 + ntok)
                    t1r = ew.tile([128, GCOL // 4], F32, tag="t1r",
                                  name="t1r")[:, :ntok]
                    nc.gpsimd.tensor_tensor(t1r, rec, t1[:, tsl], AOP.mult)
                    ctx = psCT.tile([128, 512], F32, tag="ctx",
                                    name="ctx")[:, :ncol]
                    for j, t in enumerate(tiles):
                        csl = slice(j * H * tw, (j + 1) * H * tw)
                        nc.tensor.matmul(ctx[:, csl], a_n[:, t, :],
                                         exp_a[:, csl], start=True,
                                         stop=True, skip_group_check=True)
                    ctx_sb = ew.tile([128, GCOL], BF16, tag="ctxs",
                                     name="ctxs")[:, :ncol]
                    nc.scalar.activation(ctx_sb, ctx, AFT.Copy)
                    ctxv = ctx_sb.rearrange("p (t h i) -> p t h i", h=H, i=tw)
                    x_ps = psX.tile([128, GCOL // 4], F32, tag="x",
                                    name="x")[:, :ntok]
                    for h in range(H):
                        nc.tensor.matmul(x_ps, wv_sb[:, h, :],
                                         ctxv[:, :, h, :],
                                         start=(h == 0), stop=(h == 3),
                                         skip_group_check=True)
                    # xb = x * rec * t1  (the +cv bias rides wo via wocv)
                    nc.vector.tensor_tensor(xbuf[:, tsl], x_ps, t1r, AOP.mult)

                def wo_chunk(w):
                    w0 = 480 * w
                    L = 480 if w < 4 else TOK - w0
                    for c in range(4):
                        ps_o = psO.tile([128, 512], F32, tag="o",
                                        name="o")[:, :L]
                        nc.tensor.matmul(ps_o, wo_sb[:, c, :],
                                         xbuf[:, w0:w0 + L],
                                         start=True, stop=False)
                        nc.tensor.matmul(ps_o, wocv_sb[:, c, :],
                                         t1[:, w0:w0 + L],
                                         start=False, stop=True,
                                         skip_group_check=True)
                        ot = ew.tile([128, GCOL], BF16, tag="ot",
                                     name="ot")[:, :L]
                        if c % 2:
                            nc.scalar.activation(ot, ps_o, AFT.Copy)
                        else:
                            nc.vector.tensor_copy(ot, ps_o)
                        nc.sync.dma_start(o_t[c, :, w0:w0 + L], ot)

                def prep(b):
                    """LN-apply block b's chunks and XBAR-transpose to aT."""
                    c0, c1 = b * 16, min(CH, b * 16 + 16)
                    for m, c in enumerate(range(c0, c1)):
                        src = a8s[:, c // 2, :, c % 2]
                        r = m % 16
                        if r < 2:
                            nc.vector.tensor_scalar(
                                a_n[:, c, :], src,
                                rstd_a[:, c:c + 1], nmr_a[:, c:c + 1],
                                AOP.mult, AOP.add)
                        elif r < 10:
                            nc.scalar.activation(
                                a_n[:, c, :], src, AFT.Identity,
                                bias=nmr_a[:, c:c + 1],
                                scale=rstd_a[:, c:c + 1])
                        else:
                            nc.gpsimd.tensor_scalar(
                                a_n[:, c, :], src,
                                rstd_a[:, c:c + 1], nmr_a[:, c:c + 1],
                                AOP.mult, AOP.add)
                    nc.sync.dma_start(
                        aT[:, c0:c1, :],
                        a_n[:, c0:c1, :].rearrange("p c d -> p (c d)"),
                        transpose=True)

                # software pipeline: s-phase quarters with a-stats/prep
                # interleaved; then PREP two blocks ahead of its groups,
                # STATS three ahead, a8 DMA three ahead
                for q in range(4):
                    s_quarter(q)
                # phase 1: full a pipeline; prep(b-1) overlaps stats(b)
                a_stats(0)
                for b in range(1, NBLK):
                    if b + 2 < NBLK:
                        a8_dma(b + 2)
                    a_stats(b)
                    prep(b - 1)
                prep(NBLK - 1)
                # phase 2: uninterrupted attention group stream
                r2e_fetch(0)
                r2e_fetch(1)
                g_scores(0)
                for g in range(NG_FULL + 1):
                    if g + 1 <= NG_FULL:
                        g_scores(g + 1)
                    g_rest(g)
                    if (g + 1) % 4 == 0:
                        wo_chunk((g + 1) // 4 - 1)
                wo_chunk(4)
    nc.compile()
    nc.finalize()
    return nc


def _prep(s, a, starts, counts, token_mask, w_q, w_k, w_v, w_g, w_o,
          ln_q_g, ln_q_b, ln_kv_g, ln_kv_b):
    bf = ml_dtypes.bfloat16
    sc = 1.0 / np.sqrt(np.float32(D_H))
    wq1 = ((ln_q_g[:, None] * w_q) * sc).astype(bf)
    wg1 = (ln_q_g[:, None] * w_g).astype(bf)
    wk1_t = np.asarray((ln_kv_g[:, None] * w_k).T, np.float32)
    wk1t = np.zeros((128, 4 * 128), np.float32)
    wv1_f = np.asarray(ln_kv_g[:, None] * w_v, np.float32)
    wv1 = np.zeros((128, 4 * 128), np.float32)
    for h in range(4):
        wk1t[h * 32:(h + 1) * 32, h * 128:(h + 1) * 128] = \
            wk1_t[h * 32:(h + 1) * 32, :]
        wv1[:, h * 128:(h + 1) * 128] = wv1_f * \
            (np.arange(128)[None, :] // 32 == h)
    wk1t = wk1t.astype(bf)
    wv1 = wv1.astype(bf)
    cq = ((ln_q_b @ w_q) * sc).astype(np.float32).reshape(128, 1)
    cgh = (0.5 * (ln_q_b @ w_g)).astype(np.float32).reshape(128, 1)
    cv = (ln_kv_b @ w_v).astype(np.float32)          # [128] V bias
    wo_h = (0.5 * np.asarray(w_o, np.float32)).astype(bf)
    wocv = (0.5 * cv[:, None] * np.asarray(w_o, np.float32)).astype(bf)

    jj = np.arange(128)
    ub_e = (NEG * (jj[None, :] > jj[:, None])).astype(np.float32)  # p > k
    ub_e[127, :] = NEG  # row 127 is otherwise all-zero: constant-NEG row
    ub_e = ub_e.astype(bf)

    j_tok = np.arange(TOK)
    tile_i = np.where(j_tok < NT_FULL * T, j_tok % T, j_tok - NT_FULL * T)
    col_base = np.where(j_tok < NT_FULL * T, (j_tok // T) * H * T,
                        NT_FULL * H * T)
    tw = np.where(j_tok < NT_FULL * T, T, T_TAIL)

    in_maps = []
    for c in range(NC_CORES):
        b, half = c // 2, c % 2
        n0 = half * TOK
        st = np.asarray(starts[b, n0:n0 + TOK], np.int64)
        ct = np.asarray(counts[b, n0:n0 + TOK], np.int64)
        lo = int(st[0])
        st_loc = st - lo
        assert np.all(st_loc == 8 * j_tok), "v4 premise: starts = 8n"
        assert ct.min() >= 1 and ct.max() <= W_MAX

        rows_pad = STRIDE * (CH_PAD - 1) + 128
        a_loc = np.zeros((rows_pad, 128), np.float32)
        hi = min(lo + A_ROWS, M)
        a_loc[:hi - lo] = np.asarray(a[b, lo:hi, :], np.float32)
        # pair-interleaved: a8[p, j, d, c] = a_loc[120*(2j+c) + p, d]
        idx = (STRIDE * np.arange(CH_PAD).reshape(NPAIR, 2)[None, :, :]
               + np.arange(128)[:, None, None])
        a8 = a_loc[idx].transpose(0, 1, 3, 2) \
            .reshape(128, CH_PAD * 128).astype(bf)

        s_sl = np.asarray(s[b, n0:n0 + TOK, :], np.float32)
        s8 = s_sl.reshape(16, 128, 512).transpose(1, 0, 2) \
            .reshape(128, 16 * 512).astype(bf)

        # mask = NEG*(p >= end)[end<=127] + NEG*(p < 8i), the latter as
        # const-NEG (row 127) minus NEG*(p >= 8i) via a -1 at row 8i-1
        r2 = np.zeros((128, COLS), np.float32)
        end = 8 * tile_i + ct
        for h in range(H):
            cols = col_base + h * tw + tile_i
            m_e = end <= 127
            np.add.at(r2, (np.where(m_e, end - 1, 0), cols),
                      np.where(m_e, 1.0, 0.0))
            m_s = tile_i > 0
            np.add.at(r2, (np.where(m_s, 8 * tile_i - 1, 0), cols),
                      np.where(m_s, -1.0, 0.0))
            np.add.at(r2, (np.full(TOK, 127), cols),
                      np.where(m_s, 1.0, 0.0))
        in_maps.append({
            "a8": a8, "s8": s8, "r2e": r2.astype(bf),
            "ub_e": ub_e,
            "wq1": wq1, "wg1": wg1, "wk1t": wk1t, "wv1": wv1,
            "wo": wo_h, "wocv": wocv, "cq": cq, "cgh": cgh,
        })
    return in_maps


def kernel(s, a, token_atom_starts, token_atom_counts, token_mask,
           w_q, w_k, w_v, w_g, w_o, ln_q_g, ln_q_b, ln_kv_g, ln_kv_b,
           trace=False):
    args = [np.asarray(x) for x in
            (s, a, token_atom_starts, token_atom_counts, token_mask,
             w_q, w_k, w_v, w_g, w_o, ln_q_g, ln_q_b, ln_kv_g, ln_kv_b)]
    in_maps = _prep(*args)
    if "nc" not in _cache:
        nc = bacc.Bacc(None, target_bir_lowering=False)
        _cache["nc"] = _build(nc)
    nc = _cache["nc"]
    res = run_bass_kernel_spmd(nc, in_maps, list(range(NC_CORES)),
                               trace=trace)
    out = np.zeros((B, N, D_TOK), np.float32)
    for c in range(NC_CORES):
        b, half = c // 2, c % 2
        n0 = half * TOK
        ot = np.asarray(res.results[c]["o_t"], np.float32)  # [4, 128, TOK]
        tm = np.asarray(args[4][b, n0:n0 + TOK], np.float32)
        out[b, n0:n0 + TOK, :] = ot.reshape(512, TOK).T * tm[:, None]
    kernel.last_exec_time_ns = res.exec_time_ns
    return out
